# revision 54
# baseline (speedup 1.0000x reference)
"""GCN (3-layer + global mean pool + FC/sigmoid) on 8 Trainium2 NeuronCores.

Node-sharded graph partitioning: nodes split into 8 contiguous shards of
6250; each core owns its shard's incident edges. Aggregation is
aggregate-first (A @ h, then @ W): feature rows are fetched with
dma_gather (int16 indices over two 25000-row table halves), scatter-added
via one-hot matmuls on the tensor engine. Layer 1 aggregates the 8-wide
input x directly from host-prepped padded tables (no device-side x@W1
phase). Edge slots are packed globally per stream (chunks may span two
dst tiles), with group-aligned chunk boundaries so the block schedule is
uniform across the 8 SPMD cores. Halo exchange is two AllGathers per
layer (shard halves), issued early inside the tile loop; layer L+1's
A-stream is gathered upfront into a persistent SBUF table so it never
queues behind the B-half AllGather. The head reduces pooled features
against Wfc per-core, AllGathers 512B of scalars, and assembles the
global output with shift matmuls.
"""
import sys
import os

for _p in ("/opt/trn_rl_repo", "/root/.axon_site/_ro/trn_rl_repo"):
    if os.path.isdir(_p) and _p not in sys.path:
        sys.path.append(_p)

import numpy as np
import ml_dtypes

bf16 = ml_dtypes.bfloat16

N = 50000
E = 150000
G = 256
NC = 8
SH = N // NC             # 6250 nodes per core
TPC = (SH + 127) // 128  # 49 tiles per core (last tile has 106 nodes)
HS2 = SH // 2            # 3125
HALF = N // 2            # 25000-row table halves (int16-indexable)
H1, H2, H3 = 128, 256, 512
GSZ = int(os.environ.get('KGSZ', '7'))   # tiles per chunk-alignment group
SEG = int(os.environ.get('KSEG', '8'))   # chunks per dma_gather call
NPIECE = int(os.environ.get('KNP', '2'))  # AllGather pieces per half-shard

TRACE = False
LAST_EXEC_NS = None
_CACHE = {}


def _prep(x, edge_index, edge_weight, batch):
    """Host-side graph preprocessing -> per-core metadata arrays."""
    x = np.asarray(x, np.float32)
    ei = np.asarray(edge_index)
    src = ei[0].astype(np.int64)
    dst = ei[1].astype(np.int64)
    w = np.asarray(edge_weight, np.float32)
    batch = np.asarray(batch).astype(np.int64)

    deg = np.bincount(dst, weights=w, minlength=N).astype(np.float32) + 1.0
    dinv = (1.0 / np.sqrt(deg)).astype(np.float32)
    norm = (dinv[src] * w * dinv[dst]).astype(np.float32)
    norm_self = (dinv * dinv).astype(np.float32)

    core = dst // SH
    local = dst % SH
    tile = local // 128
    srco = src // SH
    srcl = src % SH
    half = (srcl >= HS2).astype(np.int64)
    # table layout supports piece-wise AllGathers with contiguous
    # outputs: table = [piece r: 8 cores x rows PB[r]..PB[r+1]-1 of the
    # half], so src_row = f(owner, local-within-half)
    PB = [HS2 * i // NPIECE for i in range(NPIECE + 1)]
    lh_ = np.where(half == 1, srcl - HS2, srcl)
    reg = np.searchsorted(np.asarray(PB[1:]), lh_, side='right')
    pb_lo = np.asarray(PB)[reg]
    pb_sz = np.asarray(PB)[reg + 1] - pb_lo
    src_row = NC * pb_lo + srco * pb_sz + (lh_ - pb_lo)
    NG = (TPC + GSZ - 1) // GSZ
    group = tile // GSZ

    # chunk counts per (stream, group): uniform max over cores
    key_cg = (core * 2 + half) * NG + group
    cnt_chg = np.bincount(key_cg, minlength=NC * 2 * NG).reshape(NC, 2, NG)
    CHg = ((cnt_chg + 127) // 128).max(axis=0)          # [2, NG]
    chunk_base = np.zeros((2, NG + 1), np.int64)
    chunk_base[:, 1:] = np.cumsum(CHg, axis=1)
    CH = [int(CHg[0].sum()), int(CHg[1].sum())]

    # slot assignment: edges sorted by (core, half, tile, src row) — src
    # ascending within a tile segment keeps gather source addresses
    # monotonic for the DGE; each (core, half, group) run packs
    # back-to-back from its group's chunk base.
    order = np.lexsort((src_row, tile, half, core))
    sc = core[order]
    sh_ = half[order]
    sg = group[order]
    sl = local[order]
    st = tile[order]
    srow = src_row[order]
    snorm = norm[order]

    key = (sc * 2 + sh_) * NG + sg
    run_start = np.zeros(NC * 2 * NG + 1, np.int64)
    run_start[1:] = np.cumsum(np.bincount(key, minlength=NC * 2 * NG))
    offs = np.arange(len(key)) - run_start[key]
    slot = chunk_base[(sh_, sg)] * 128 + offs
    kchunk = slot // 128
    spos = slot % 128

    # per-(stream, tile) chunk spans: min/max over all cores' edges
    k_first = np.full((2, TPC), 1 << 30, np.int64)
    k_last = np.full((2, TPC), -1, np.int64)
    np.minimum.at(k_first, (sh_, st), kchunk)
    np.maximum.at(k_last, (sh_, st), kchunk)

    # oh block layout: per tile t: [A-span blocks][B-span blocks][self]
    CHmax = max(CH)
    colmap = np.full((2, CHmax, TPC), -1, np.int64)
    sched = []
    selfcol = np.zeros(TPC, np.int64)
    col = 0
    for t in range(TPC):
        ent = []
        for h in (0, 1):
            if k_last[h, t] >= 0:
                for k in range(int(k_first[h, t]), int(k_last[h, t]) + 1):
                    colmap[h, k, t] = col
                    ent.append((h, int(k), col))
                    col += 128
        selfcol[t] = col
        col += 128
        sched.append(ent)
    BLK = col // 128

    cntg = np.bincount(batch, minlength=G).astype(np.float32)
    cntinv_g = (1.0 / np.maximum(cntg, 1.0)).astype(np.float32)
    g0s = [int(batch[c * SH]) for c in range(NC)]
    for c in range(NC):
        assert int(batch[(c + 1) * SH - 1]) - g0s[c] < 128, "graph window > 128"

    # head shift blocks: nonzero (core, graph-half) pairs
    head_blocks = []
    shift_mats = []
    for c in range(NC):
        for gh in range(2):
            pd = g0s[c] + np.arange(128) - gh * 128
            val = (pd >= 0) & (pd < 128) & (g0s[c] + np.arange(128) < G)
            if not val.any():
                continue
            m = np.zeros((128, 128), np.float32)
            rows = np.arange(128)[val]
            m[rows, pd[val]] = 1.0
            head_blocks.append((c, gh, len(shift_mats) * 128))
            shift_mats.append(m)
    NSH = len(shift_mats)
    shift_all = np.concatenate(shift_mats, axis=1)  # [128, NSH*128]

    def idx_pack(lin):
        a = lin.reshape(-1, 16).T
        return np.ascontiguousarray(np.tile(a, (8, 1)))

    norm_b = snorm.astype(bf16).astype(np.float32)
    per_core = []
    for c in range(NC):
        m = sc == c
        idxs = []
        for h in (0, 1):
            mh = m & (sh_ == h)
            ia = np.zeros(CH[h] * 128, np.int16)
            ia[slot[mh]] = srow[mh].astype(np.int16)
            idxs.append(idx_pack(ia))
        oh = np.zeros((128, BLK * 128), np.float32)
        cols = colmap[(sh_[m], kchunk[m], st[m])] + (sl[m] % 128)
        assert (colmap[(sh_[m], kchunk[m], st[m])] >= 0).all()
        oh[spos[m], cols] = norm_b[m]
        # self-loop diagonal blocks
        pr = np.arange(128)
        for t in range(TPC):
            nrows = min(128, SH - t * 128)
            nodes = c * SH + t * 128 + pr[:nrows]
            oh[pr[:nrows], selfcol[t] + pr[:nrows]] = norm_self[nodes]

        xs = np.zeros((128, TPC * 8), np.float32)
        for t in range(TPC):
            nrows = min(128, SH - t * 128)
            xs[:nrows, t * 8:(t + 1) * 8] = x[c * SH + t * 128:
                                              c * SH + t * 128 + nrows]

        bl = np.full((TPC * 128,), -1.0, np.float32)
        bl[:SH] = (batch[c * SH:(c + 1) * SH] - g0s[c]).astype(np.float32)
        ig = g0s[c] + np.arange(128)
        cinv = np.where(ig < G, cntinv_g[np.minimum(ig, G - 1)], 0.0)

        # static pool one-hot: poolsel[p, t*128+g] = (graph-local of node
        # (c,t,p) == g)
        blv = bl.reshape(TPC, 128)
        psel = np.zeros((128, TPC, 128), np.float32)
        pr2 = np.arange(128)
        for t in range(TPC):
            vm = blv[t] >= 0
            psel[pr2[vm], t, blv[t][vm].astype(np.int64)] = 1.0

        per_core.append(dict(
            idxA=idxs[0], idxB=idxs[1],
            ohall=np.ascontiguousarray(oh).astype(bf16),
            xself=xs.astype(bf16),
            poolsel=np.ascontiguousarray(psel.reshape(128, TPC * 128)).astype(bf16),
            cntinv=cinv.astype(np.float32).reshape(128, 1),
        ))

    # shared padded-x gather tables in piece layout, 8 -> 128 cols
    r = np.arange(HALF)
    rreg = np.searchsorted(np.asarray([NC * b for b in PB[1:]]), r,
                           side='right')
    rlo = np.asarray(PB)[rreg]
    rsz = np.asarray(PB)[rreg + 1] - rlo
    owner = (r - NC * rlo) // rsz
    loc_h = rlo + (r - NC * rlo) % rsz
    xpads = []
    for h in (0, 1):
        xp = np.zeros((HALF, 128), np.float32)
        xp[:, :8] = x[owner * SH + h * HS2 + loc_h]
        xpads.append(xp.astype(bf16))

    struct = dict(
        CHA=CH[0], CHB=CH[1], BLK=BLK, NSH=NSH,
        sched=tuple(tuple(e) for e in sched),
        selfcol=tuple(int(v) for v in selfcol),
        head_blocks=tuple(head_blocks),
        g0s=tuple(g0s),
    )
    shared_host = dict(xA=xpads[0], xB=xpads[1],
                       shiftm=shift_all.astype(bf16))
    return per_core, shared_host, struct


def _build(struct):
    import concourse.bacc as bacc
    import concourse.mybir as mybir
    import concourse.tile as tile
    from concourse.masks import make_identity

    dt = mybir.dt
    AF = mybir.ActivationFunctionType
    OP = mybir.AluOpType

    CHA, CHB = struct["CHA"], struct["CHB"]
    BLK, NSH = struct["BLK"], struct["NSH"]
    sched = struct["sched"]
    selfcol = struct["selfcol"]
    head_blocks = struct["head_blocks"]
    CHs = (CHA, CHB)

    nc = bacc.Bacc("TRN2", target_bir_lowering=False, debug=False,
                   num_devices=NC)

    xA_in = nc.dram_tensor("xA", [HALF, 128], dt.bfloat16, kind="ExternalInput")
    xB_in = nc.dram_tensor("xB", [HALF, 128], dt.bfloat16, kind="ExternalInput")
    w1_in = nc.dram_tensor("w1", [8, H1], dt.bfloat16, kind="ExternalInput")
    w2_in = nc.dram_tensor("w2", [H1, H2], dt.bfloat16, kind="ExternalInput")
    w3_in = nc.dram_tensor("w3", [128, 2, H3], dt.bfloat16, kind="ExternalInput")
    wfc_in = nc.dram_tensor("wfc", [128, 4], dt.float32, kind="ExternalInput")
    bbc_in = nc.dram_tensor("bbc", [128, H1 + H2 + H3], dt.bfloat16,
                            kind="ExternalInput")
    bfc_in = nc.dram_tensor("bfc", [1, 1], dt.float32, kind="ExternalInput")
    idxA_in = nc.dram_tensor("idxA", [128, CHA * 8], dt.int16, kind="ExternalInput")
    idxB_in = nc.dram_tensor("idxB", [128, CHB * 8], dt.int16, kind="ExternalInput")
    oh_in = nc.dram_tensor("ohall", [128, BLK * 128], dt.bfloat16,
                           kind="ExternalInput")
    xself_in = nc.dram_tensor("xself", [128, TPC * 8], dt.bfloat16,
                              kind="ExternalInput")
    psel_in = nc.dram_tensor("poolsel", [128, TPC * 128], dt.bfloat16,
                             kind="ExternalInput")
    cinv_in = nc.dram_tensor("cntinv", [128, 1], dt.float32, kind="ExternalInput")
    shift_in = nc.dram_tensor("shiftm", [128, NSH * 128], dt.bfloat16,
                              kind="ExternalInput")
    out_ext = nc.dram_tensor("out", [G, 1], dt.float32, kind="ExternalOutput")

    with tile.TileContext(nc) as tc:
        with tc.tile_pool(name="const", bufs=1) as cp, \
             tc.tile_pool(name="meta", bufs=1) as mp, \
             tc.tile_pool(name="work", bufs=3) as wp, \
             tc.tile_pool(name="slabs", bufs=1) as slp, \
             tc.tile_pool(name="pps", bufs=1, space="PSUM") as pps, \
             tc.tile_pool(name="dram", bufs=1, space="DRAM") as dram:

            def load(pool, t_in, shape, dtype, tag):
                t = pool.tile(shape, dtype, tag=tag)
                nc.sync.dma_start(t[:], t_in[:])
                return t

            idx_sbs = [load(mp, idxA_in, [128, CHA * 8], dt.int16, "idxA"),
                       load(mp, idxB_in, [128, CHB * 8], dt.int16, "idxB")]
            w1_sb = load(cp, w1_in, [8, H1], dt.bfloat16, "w1")
            w2_sb = load(cp, w2_in, [H1, H2], dt.bfloat16, "w2")
            w3_sb = load(cp, w3_in, [128, 2, H3], dt.bfloat16, "w3")
            wfc_sb = load(cp, wfc_in, [128, 4], dt.float32, "wfc")
            bbc_sb = load(cp, bbc_in, [128, H1 + H2 + H3], dt.bfloat16, "bbc")
            bfc_sb = load(cp, bfc_in, [1, 1], dt.float32, "bfc")
            shift_sb = load(cp, shift_in, [128, NSH * 128], dt.bfloat16,
                            "shiftm")
            # one-hot table: load in column slices so early tiles don't wait
            # on the full 10+ MB transfer
            oh_all = mp.tile([128, BLK * 128], dt.bfloat16, tag="ohall")
            NSL = 8
            slw = ((BLK + NSL - 1) // NSL) * 128
            for s in range(NSL):
                c0 = s * slw
                c1 = min(BLK * 128, c0 + slw)
                if c0 < c1:
                    nc.sync.dma_start(oh_all[:, c0:c1], oh_in[:, c0:c1])
            psel_sb = load(mp, psel_in, [128, TPC * 128], dt.bfloat16, "psel")
            cinv_sb = load(mp, cinv_in, [128, 1], dt.float32, "cinv")

            ones_f32 = cp.tile([1, 128], dt.float32, tag="ones_f32")
            nc.vector.memset(ones_f32[:], 1.0)
            ones_bf = cp.tile([1, 128], dt.bfloat16, tag="ones_bf")
            nc.vector.memset(ones_bf[:], 1.0)
            ident = cp.tile([128, 128], dt.float32, tag="ident")
            make_identity(nc, ident[:])

            slab1 = slp.tile([128, TPC, 8], dt.bfloat16, tag="slab1")
            nc.sync.dma_start(slab1[:], xself_in[:].rearrange(
                "p (t f) -> p t f", f=8))
            slab2 = slp.tile([128, TPC, H1], dt.bfloat16, tag="slab2")
            nc.vector.memset(slab2[:, TPC - 1, :], 0.0)
            slab3 = slp.tile([128, TPC, H2], dt.bfloat16, tag="slab3")
            nc.vector.memset(slab3[:, TPC - 1, :], 0.0)
            slabs = {1: slab1, 2: slab2, 3: slab3}

            h1_shard = dram.tile([SH, H1], dt.bfloat16, tag="h1s")
            h1A = dram.tile([HALF, H1], dt.bfloat16, tag="h1A")
            h1B = dram.tile([HALF, H1], dt.bfloat16, tag="h1B")
            h2_shard = dram.tile([SH, H2], dt.float8e4, tag="h2s")
            h2A = dram.tile([HALF, H2], dt.float8e4, tag="h2A")
            h2B = dram.tile([HALF, H2], dt.float8e4, tag="h2B")
            s_shard = dram.tile([128, 1], dt.bfloat16, tag="ss")
            s_all = dram.tile([NC * 128, 1], dt.bfloat16, tag="sa")

            PBv = [HS2 * i // NPIECE for i in range(NPIECE + 1)]
            PIECES = []
            for hb in range(2):
                for r in range(NPIECE):
                    r0 = hb * HS2 + PBv[r]
                    r1 = hb * HS2 + PBv[r + 1]
                    pt = min(TPC - 1, (r1 - 1) // 128)
                    PIECES.append((pt, r0, r1, hb,
                                   NC * PBv[r], NC * PBv[r + 1]))

            def issue_ags(t, out_shard, tabAB):
                if tabAB[0] is None:
                    return
                for (pt, r0, r1, hb, o0, o1) in PIECES:
                    if t == pt:
                        ag(out_shard[r0:r1, :], tabAB[hb][o0:o1, :])

            def ag(in_ap, out_t):
                nc.gpsimd.collective_compute(
                    "AllGather", mybir.AluOpType.bypass,
                    replica_groups=[list(range(NC))],
                    ins=[in_ap.opt() if hasattr(in_ap, 'opt') else in_ap],
                    outs=[out_t.opt() if hasattr(out_t, 'opt') else out_t])

            pool_ps = pps.tile([128, H3], dt.float32)

            def mk_seg(lidx, tabs, elem, gp, seg_tiles):
                gdt = dt.float8e4 if lidx == 3 else dt.bfloat16

                def seg(stream, s):
                    key = (stream, s)
                    if key not in seg_tiles:
                        nch = min(SEG, CHs[stream] - s * SEG)
                        t = gp.tile([128, SEG, elem], gdt,
                                    tag=f"g{stream}", bufs=2)
                        nc.gpsimd.dma_gather(
                            t[:, :nch, :], tabs[stream][:, :],
                            idx_sbs[stream][:, s * SEG * 8:(s * SEG + nch) * 8],
                            nch * 128, nch * 128, elem,
                            single_packet=False)
                        seg_tiles[key] = t
                    return seg_tiles[key]
                return seg

            def do_layer1(out_shard, agA, agB, gp):
                seg = mk_seg(1, (xA_in, xB_in), 128, gp, {})
                with tc.tile_pool(name="psl1", bufs=3, space="PSUM") as psl:
                    for t in range(TPC):
                        rows = min(128, SH - t * 128)
                        chain = [(2, t, selfcol[t])] + list(sched[t])
                        agg = psl.tile([8, 128], dt.float32, tag="agg0")
                        for i, (stm, k, col) in enumerate(chain):
                            lh = (slab1[:, k, :] if stm == 2
                                  else seg(stm, k // SEG)[:, k % SEG, :8])
                            nc.tensor.matmul(
                                agg[:], lhsT=lh,
                                rhs=oh_all[:, col:col + 128],
                                start=(i == 0), stop=(i == len(chain) - 1))
                        aggx_sb = wp.tile([8, 128], dt.bfloat16, tag="aggx")
                        if t % 2 == 0:
                            nc.vector.tensor_copy(aggx_sb[:], agg[:])
                        else:
                            nc.scalar.activation(aggx_sb[:], agg[:], AF.Copy)
                        h_ps = psl.tile([128, H1], dt.float32, tag="hps")
                        nc.tensor.matmul(h_ps[:], lhsT=ones_bf[:],
                                         rhs=bbc_sb[0:1, :H1], start=True,
                                         stop=False)
                        nc.tensor.matmul(h_ps[:], lhsT=aggx_sb[:],
                                         rhs=w1_sb[:], start=False, stop=True)
                        nc.scalar.activation(slab2[:rows, t, :], h_ps[:rows],
                                             AF.Relu)
                        nc.sync.dma_start(
                            out_shard[t * 128:t * 128 + rows, :],
                            slab2[:rows, t, :])
                        issue_ags(t, out_shard, (agA, agB))

            def do_layer23(lidx, tabs, elem, fcn, out_shard, agA, agB, gp):
                fout = H2 if lidx == 2 else H3
                w_rhs = (lambda fc: w2_sb[:]) if lidx == 2 else \
                        (lambda fc: w3_sb[:, fc, :])
                bc0, bc1 = (H1, H1 + H2) if lidx == 2 else \
                           (H1 + H2, H1 + H2 + H3)
                seg = mk_seg(lidx, tabs, elem, gp, {})
                slab = slabs[lidx]
                aggA_sbs = {}
                # loop 1: self + A-stream accumulation for every tile (PE
                # stays busy while the B-half AllGather is in flight)
                with tc.tile_pool(name=f"psa{lidx}", bufs=3, space="PSUM") as psa:
                    for t in range(TPC):
                        chainA = [(2, t, selfcol[t])] + \
                                 [e for e in sched[t] if e[0] == 0]
                        aggs = [psa.tile([128, 128], dt.float32,
                                         tag=f"aggA{fc}", name=f"aggA{fc}")
                                for fc in range(fcn)]
                        for i, (stm, k, col) in enumerate(chainA):
                            for fc in range(fcn):
                                lh = (slab[:, k, fc * 128:(fc + 1) * 128]
                                      if stm == 2 else
                                      seg(stm, k // SEG)[:, k % SEG,
                                                         fc * 128:(fc + 1) * 128])
                                nc.tensor.matmul(
                                    aggs[fc][:], lhsT=lh,
                                    rhs=oh_all[:, col:col + 128],
                                    start=(i == 0), stop=(i == len(chainA) - 1))
                        for fc in range(fcn):
                            a = gp.tile([128, 128], dt.bfloat16,
                                        tag=f"asb{t}_{fc}", bufs=1)
                            nc.scalar.activation(a[:], aggs[fc][:], AF.Copy)
                            aggA_sbs[(t, fc)] = a
                # loop 2: B-stream + weight matmul + output
                with tc.tile_pool(name=f"psb{lidx}", bufs=3, space="PSUM") as psb:
                    for t in range(TPC):
                        rows = min(128, SH - t * 128)
                        chainB = [e for e in sched[t] if e[0] == 1]
                        asums = []
                        if chainB:
                            aggs = [psb.tile([128, 128], dt.float32,
                                             tag=f"aggB{fc}", name=f"aggB{fc}",
                                             bufs=2)
                                    for fc in range(fcn)]
                            for i, (stm, k, col) in enumerate(chainB):
                                for fc in range(fcn):
                                    lh = seg(stm, k // SEG)[:, k % SEG,
                                                            fc * 128:(fc + 1) * 128]
                                    nc.tensor.matmul(
                                        aggs[fc][:], lhsT=lh,
                                        rhs=oh_all[:, col:col + 128],
                                        start=(i == 0),
                                        stop=(i == len(chainB) - 1))
                            for fc in range(fcn):
                                a = wp.tile([128, 128], dt.bfloat16,
                                            tag=f"bsum{fc}", name=f"bsum{fc}")
                                nc.vector.tensor_tensor(
                                    a[:], aggs[fc][:],
                                    aggA_sbs[(t, fc)][:], OP.add)
                                asums.append(a)
                        else:
                            asums = [aggA_sbs[(t, fc)] for fc in range(fcn)]
                        h_ps = psb.tile([128, fout], dt.float32, tag="hps")
                        nc.tensor.matmul(h_ps[:], lhsT=ones_bf[:],
                                         rhs=bbc_sb[0:1, bc0:bc1], start=True,
                                         stop=False)
                        for fc in range(fcn):
                            nc.tensor.matmul(h_ps[:], lhsT=asums[fc][:],
                                             rhs=w_rhs(fc), start=False,
                                             stop=(fc == fcn - 1))
                        if lidx == 2:
                            nc.scalar.activation(slab3[:rows, t, :],
                                                 h_ps[:rows], AF.Relu)
                            h8 = wp.tile([128, H2], dt.float8e4, tag="h8")
                            nc.vector.tensor_copy(h8[:rows], slab3[:rows, t, :])
                            nc.sync.dma_start(
                                out_shard[t * 128:t * 128 + rows, :],
                                h8[:rows])
                        else:
                            h_sb = wp.tile([128, H3], dt.bfloat16, tag="hsb")
                            nc.scalar.activation(h_sb[:], h_ps[:], AF.Relu)
                            nc.tensor.matmul(
                                pool_ps[:],
                                lhsT=psel_sb[:, t * 128:(t + 1) * 128],
                                rhs=h_sb[:], start=(t == 0),
                                stop=(t == TPC - 1))
                        issue_ags(t, out_shard, (agA, agB))

            with tc.tile_pool(name="gl1", bufs=2) as gp1:
                do_layer1(h1_shard, h1A, h1B, gp1)
            with tc.tile_pool(name="gl2", bufs=2) as gp2:
                do_layer23(2, (h1A, h1B), 128, 1, h2_shard, h2A, h2B, gp2)
            with tc.tile_pool(name="gl3", bufs=2) as gp3:
                do_layer23(3, (h2A, h2B), 256, 2, None, None, None, gp3)

            # ---- head: pooled/cnt -> @Wfc per-core -> 512B AG -> assemble ----
            with tc.tile_pool(name="psf", bufs=2, space="PSUM") as psf:
                pool_sb = wp.tile([128, H3], dt.float32, tag="poolsb")
                nc.vector.tensor_scalar(pool_sb[:], pool_ps[:],
                                        cinv_sb[:, :1], None, OP.mult)
                s_ps = psf.tile([128, 1], dt.float32, tag="sps")
                for fc in range(4):
                    tr_ps = psf.tile([128, 128], dt.float32, tag="tr", bufs=2)
                    nc.tensor.transpose(tr_ps[:],
                                        pool_sb[:, fc * 128:(fc + 1) * 128],
                                        ident[:])
                    ptf = wp.tile([128, 128], dt.float32, tag="ptf", bufs=2)
                    nc.vector.tensor_copy(ptf[:], tr_ps[:])
                    nc.tensor.matmul(s_ps[:], lhsT=ptf[:],
                                     rhs=wfc_sb[:, fc:fc + 1],
                                     start=(fc == 0), stop=(fc == 3))
                s_sb = wp.tile([128, 1], dt.bfloat16, tag="ssb")
                nc.scalar.activation(s_sb[:], s_ps[:], AF.Copy)
                nc.sync.dma_start(s_shard[:], s_sb[:])
                ag(s_shard, s_all)

                sall_sb = wp.tile([128, NC], dt.bfloat16, tag="sall")
                nc.sync.dma_start(sall_sb[:],
                                  s_all[:].rearrange("(c p) o -> p (c o)", p=128))
                for gh in range(2):
                    blks = [hb for hb in head_blocks if hb[1] == gh]
                    o_ps = psf.tile([128, 1], dt.float32, tag="ops", bufs=2)
                    nc.tensor.matmul(o_ps[:], lhsT=ones_f32[:], rhs=bfc_sb[:],
                                     start=True, stop=(len(blks) == 0))
                    for i, (c, _, scol) in enumerate(blks):
                        nc.tensor.matmul(o_ps[:],
                                         lhsT=shift_sb[:, scol:scol + 128],
                                         rhs=sall_sb[:, c:c + 1],
                                         start=False, stop=(i == len(blks) - 1))
                    o_sb = wp.tile([128, 1], dt.float32, tag="osb", bufs=2)
                    nc.scalar.activation(o_sb[:], o_ps[:], AF.Sigmoid)
                    nc.sync.dma_start(out_ext[gh * 128:(gh + 1) * 128, :],
                                      o_sb[:])

    nc.compile()
    return nc


def _install_profile_hook():
    """Provide antenv.axon_hooks (NTFF profiling) if the image lacks it."""
    import importlib
    try:
        importlib.import_module("antenv.axon_hooks")
        return
    except ImportError:
        pass
    import types
    import ctypes
    import contextlib
    so_path = "/opt/axon/libaxon_pjrt.so"
    mod = types.ModuleType("antenv.axon_hooks")
    _state = {"hook": None}

    def set_axon_ntff_profile_hook(h):
        _state["hook"] = h

    def get_axon_ntff_profile_hook():
        if _state["hook"] is None and os.path.exists(so_path):
            lib = ctypes.CDLL(so_path)
            if hasattr(lib, "axon_start_nrt_profile"):
                lib.axon_start_nrt_profile.argtypes = [
                    ctypes.POINTER(ctypes.c_int64), ctypes.c_size_t]
                lib.axon_start_nrt_profile.restype = ctypes.c_int64
                lib.axon_stop_nrt_profile.argtypes = [ctypes.c_char_p]
                lib.axon_stop_nrt_profile.restype = ctypes.c_int64

                @contextlib.contextmanager
                def _hook(output_dir, device_ids):
                    import jax
                    jax.devices()
                    if device_ids:
                        ids = (ctypes.c_int64 * len(device_ids))(*device_ids)
                        rc = lib.axon_start_nrt_profile(ids, len(device_ids))
                    else:
                        rc = lib.axon_start_nrt_profile(None, 0)
                    if rc != 0:
                        raise RuntimeError(f"axon_start_nrt_profile rc={rc}")
                    try:
                        yield
                    finally:
                        n = lib.axon_stop_nrt_profile(str(output_dir).encode())
                        print(f"profile: {n} file(s) written to {output_dir}")

                _state["hook"] = _hook
        return _state["hook"]

    mod.set_axon_ntff_profile_hook = set_axon_ntff_profile_hook
    mod.get_axon_ntff_profile_hook = get_axon_ntff_profile_hook
    sys.modules["antenv.axon_hooks"] = mod


def kernel(**inputs):
    global LAST_EXEC_NS
    from concourse.bass_utils import run_bass_kernel_spmd

    per_core, shared_host, struct = _prep(
        inputs["x"], inputs["edge_index"], inputs["edge_weight"],
        inputs["batch"])

    key = (struct["CHA"], struct["CHB"], struct["BLK"], struct["sched"],
           struct["selfcol"], struct["head_blocks"], struct["g0s"])
    if key not in _CACHE:
        _CACHE[key] = _build(struct)
    nc = _CACHE[key]

    W1 = np.asarray(inputs["W1"], np.float32)
    W2 = np.asarray(inputs["W2"], np.float32)
    W3 = np.asarray(inputs["W3"], np.float32)
    Wfc = np.asarray(inputs["Wfc"], np.float32)
    shared = dict(
        xA=shared_host["xA"],
        xB=shared_host["xB"],
        shiftm=shared_host["shiftm"],
        w1=W1.astype(bf16),
        w2=W2.astype(bf16),
        w3=np.ascontiguousarray(
            W3.reshape(2, 128, H3).transpose(1, 0, 2)).astype(bf16),
        wfc=np.ascontiguousarray(Wfc.reshape(4, 128).T).astype(np.float32),
        bbc=np.ascontiguousarray(np.tile(np.concatenate([
            np.asarray(inputs["b1"], np.float32).reshape(-1),
            np.asarray(inputs["b2"], np.float32).reshape(-1),
            np.asarray(inputs["b3"], np.float32).reshape(-1),
        ])[None, :], (128, 1))).astype(bf16),
        bfc=np.asarray(inputs["bfc"], np.float32).reshape(1, 1),
    )
    in_maps = [{**shared, **pc} for pc in per_core]

    if TRACE:
        _install_profile_hook()
    res = run_bass_kernel_spmd(nc, in_maps, list(range(NC)), trace=TRACE)
    LAST_EXEC_NS = res.exec_time_ns
    return res.results[0]["out"]
